# revision 1
# baseline (speedup 1.0000x reference)
"""Multi-scale deformable attention on 8 Trainium2 NeuronCores.

Sharding: batch dim (8 batch elements -> 8 cores, data parallel).

Per-core pipeline:
  P1: v' = value @ w_val + b_val  -> per-head bf16 pixel tables in DRAM
      (one zero pad row at each end of the flattened-per-head table).
  P2: off|attn logits = query @ [w_off|w_attn] + bias (PE, rank-1 fp32 bias).
  P3: DVE: sampling coords (shifted +2 so mod-floor is sign-safe), corner
      validity masks, bilinear*attention weights, int32 gather row indices
      (head table base folded into a host-fed constant).
  P4: gpsimd indirect DMA gathers 2-pixel spans (64 elems) per (q,h,l,p,y);
      DVE multiply by broadcast weights + strided tensor_reduce over the 64
      (l,p,y,px) slots -> out[q, h, 32].
  P5: PE-transpose + matmul w_out + bias -> DMA out.
"""

import os
import sys

import numpy as np

if "/opt/trn_rl_repo" not in sys.path:
    sys.path.insert(0, "/opt/trn_rl_repo")
if "/opt/pypackages" not in sys.path:
    sys.path.append("/opt/pypackages")

# ---------------------------------------------------------------- problem cfg
SPATIAL_SHAPES = [(128, 128), (64, 64), (32, 32), (16, 16)]
C = 256
NH = 8
NL = 4
NP = 4
HD = C // NH          # 32
BS = 8
NQ = 2000
NQP = 2048            # query count padded to a multiple of 128
NV = sum(h * w for h, w in SPATIAL_SHAPES)   # 21760
NVT = NV + 2          # per-head table rows incl. front/back zero pad row
P = 128
NQB = NQP // P        # 16 query blocks
NOFF = NH * NL * NP * 2   # 256 offset channels
NATT = NH * NL * NP       # 128 attention logits
HLP = NH * NL * NP        # 128
HGRP = 4                  # heads gathered per indirect DMA
LEVEL_BASE = [0, 16384, 20480, 21504]

_CACHE = {}


# ================================================================ bass module
def _build_module():
    import concourse.bacc as bacc
    import concourse.bass as bass
    import concourse.mybir as mybir
    import concourse.tile as tile
    from concourse.masks import make_identity

    FP32 = mybir.dt.float32
    BF16 = mybir.dt.bfloat16
    I32 = mybir.dt.int32
    MULT = mybir.AluOpType.mult
    ADD = mybir.AluOpType.add
    SUB = mybir.AluOpType.subtract
    MAXO = mybir.AluOpType.max
    MINO = mybir.AluOpType.min
    GE = mybir.AluOpType.is_ge
    LE = mybir.AluOpType.is_le
    ACT_COPY = mybir.ActivationFunctionType.Copy
    ACT_EXP = mybir.ActivationFunctionType.Exp
    ACT_RELU = mybir.ActivationFunctionType.Relu

    nc = bacc.Bacc("TRN2", target_bir_lowering=False, debug=False)

    # ------------------------------------------------ external inputs (1 core)
    value = nc.dram_tensor("value", [NV, C], FP32, kind="ExternalInput")
    query = nc.dram_tensor("query", [NQP, C], FP32, kind="ExternalInput")
    refp = nc.dram_tensor("refp", [NQP, NP * 2], FP32, kind="ExternalInput")
    wval = nc.dram_tensor("wval", [C, C], BF16, kind="ExternalInput")
    bval = nc.dram_tensor("bval", [1, C], FP32, kind="ExternalInput")
    wcat = nc.dram_tensor("wcat", [C, NOFF + NATT], BF16, kind="ExternalInput")
    bcat = nc.dram_tensor("bcat", [1, NOFF + NATT], FP32, kind="ExternalInput")
    wout = nc.dram_tensor("wout", [C, C], BF16, kind="ExternalInput")
    bout = nc.dram_tensor("bout", [1, C], FP32, kind="ExternalInput")
    # constants, pre-replicated across partitions host-side
    dims = nc.dram_tensor("dims", [P, NOFF], FP32, kind="ExternalInput")
    dimsp1 = nc.dram_tensor("dimsp1", [P, NOFF], FP32, kind="ExternalInput")
    dims2 = nc.dram_tensor("dims2", [P, NOFF], FP32, kind="ExternalInput")
    lowc = nc.dram_tensor("lowc", [P, NOFF], FP32, kind="ExternalInput")
    wxc = nc.dram_tensor("wxc", [P, HLP], FP32, kind="ExternalInput")
    kkc = nc.dram_tensor("kkc", [P, HLP], FP32, kind="ExternalInput")
    onesr = nc.dram_tensor("onesr", [1, P], FP32, kind="ExternalInput")

    out = nc.dram_tensor("out", [NQP, C], FP32, kind="ExternalOutput")

    # ------------------------------------------------ internal DRAM scratch
    vbf = nc.dram_tensor("vbf", [NV, C], BF16)
    vtbl = nc.dram_tensor("vtbl", [NH, NVT, 2 * HD], BF16)

    XCHUNK = 2048  # xbar transpose chunk (pixels)

    with tile.TileContext(nc) as tc:
        cpool = tc.alloc_tile_pool(name="consts", bufs=1)
        sb = tc.alloc_tile_pool(name="sb", bufs=2)
        sb3 = tc.alloc_tile_pool(name="sb3", bufs=3)
        pg = tc.alloc_tile_pool(name="pg", bufs=2)  # gather dests
        pp1 = tc.alloc_tile_pool(name="pp1", bufs=2, space="PSUM")

        # ---------------- constants to SBUF
        t_wval = cpool.tile([P, 2, C], BF16)      # [k-half, khalf-idx, out]
        nc.sync.dma_start(out=t_wval[:], in_=wval[:].rearrange("(a k) n -> k a n", a=2))
        t_wcat = cpool.tile([P, 2, NOFF + NATT], BF16)
        nc.sync.dma_start(out=t_wcat[:], in_=wcat[:].rearrange("(a k) n -> k a n", a=2))
        t_wout = cpool.tile([P, 2, C], BF16)
        nc.sync.dma_start(out=t_wout[:], in_=wout[:].rearrange("(a k) n -> k a n", a=2))
        t_bval = cpool.tile([1, C], FP32)
        nc.sync.dma_start(out=t_bval[:], in_=bval[:])
        t_bcat = cpool.tile([1, NOFF + NATT], FP32)
        nc.sync.dma_start(out=t_bcat[:], in_=bcat[:])
        t_bout = cpool.tile([1, C], FP32)
        nc.sync.dma_start(out=t_bout[:], in_=bout[:])
        t_dims = cpool.tile([P, NOFF], FP32)
        nc.sync.dma_start(out=t_dims[:], in_=dims[:])
        t_dimsp1 = cpool.tile([P, NOFF], FP32)
        nc.sync.dma_start(out=t_dimsp1[:], in_=dimsp1[:])
        t_dims2 = cpool.tile([P, NOFF], FP32)
        nc.sync.dma_start(out=t_dims2[:], in_=dims2[:])
        t_lowc = cpool.tile([P, NOFF], FP32)
        nc.sync.dma_start(out=t_lowc[:], in_=lowc[:])
        t_wx = cpool.tile([P, HLP], FP32)
        nc.sync.dma_start(out=t_wx[:], in_=wxc[:])
        t_kk = cpool.tile([P, HLP], FP32)
        nc.sync.dma_start(out=t_kk[:], in_=kkc[:])
        t_ones = cpool.tile([1, P], FP32)
        nc.sync.dma_start(out=t_ones[:], in_=onesr[:])
        ident = cpool.tile([P, P], BF16)
        make_identity(nc, ident[:])

        # ---------------- P1a: cast value fp32 -> bf16 in DRAM
        for i in range(NV // P):
            vc = sb3.tile([P, C], BF16, tag="vcast")
            nc.gpsimd.dma_start(out=vc[:], in_=value[i * P:(i + 1) * P, :])
            nc.sync.dma_start(out=vbf[i * P:(i + 1) * P, :], in_=vc[:])

        # ---------------- P1b: transpose chunks, project, write tables
        # zero pad rows (front/back) of each head table
        zpad = cpool.tile([NH, 3, 2 * HD], BF16)
        nc.vector.memset(zpad[:], 0)
        pad_view = bass.AP(
            vtbl, 0,
            [[NVT * 2 * HD, NH], [NV * 2 * HD, 2], [1, 2 * HD]],
        )
        nc.sync.dma_start(out=pad_view, in_=zpad[:, :2, :])
        pad_view0 = bass.AP(vtbl, 0, [[NVT * 2 * HD, NH], [1, 2 * HD]])
        nc.sync.dma_start(out=pad_view0, in_=zpad[:, 2, :])

        off_px = 0
        while off_px < NV:
            chunk = min(XCHUNK, NV - off_px)
            vT = sb.tile([P, 2, XCHUNK], BF16, tag="vT")
            for half in range(2):
                nc.sync.dma_start_transpose(
                    out=vT[:, half, :chunk],
                    in_=vbf[off_px:off_px + chunk, half * P:(half + 1) * P],
                )
            for m in range(chunk // P):
                ps = pp1.tile([P, C], FP32, space="PSUM", tag="p1ps")
                nc.tensor.matmul(
                    out=ps[:], lhsT=vT[:, 0, m * P:(m + 1) * P],
                    rhs=t_wval[:, 0, :], start=True, stop=False)
                nc.tensor.matmul(
                    out=ps[:], lhsT=vT[:, 1, m * P:(m + 1) * P],
                    rhs=t_wval[:, 1, :], start=False, stop=False)
                nc.tensor.matmul(
                    out=ps[:], lhsT=t_ones[:, :], rhs=t_bval[:],
                    start=False, stop=True)
                ev = sb3.tile([P, C], BF16, tag="p1ev")
                nc.scalar.activation(out=ev[:], in_=ps[:], func=ACT_COPY)
                row0 = off_px + m * P
                evh = ev[:].rearrange("p (h c) -> p h c", h=NH)
                dst1 = bass.AP(
                    vtbl, (1 + row0) * 2 * HD,
                    [[2 * HD, P], [NVT * 2 * HD, NH], [1, HD]],
                )
                nc.sync.dma_start(out=dst1, in_=evh)
                dst2 = bass.AP(
                    vtbl, row0 * 2 * HD + HD,
                    [[2 * HD, P], [NVT * 2 * HD, NH], [1, HD]],
                )
                nc.sync.dma_start(out=dst2, in_=evh)
            off_px += chunk

        # flat per-head table view for gathers: [NH*NVT, HD], offset 0
        vtbl_flat = bass.AP(vtbl, 0, [[2 * HD, NH * NVT], [1, 2 * HD]])

        pp1.release()
        pp = tc.alloc_tile_pool(name="pp", bufs=2, space="PSUM")

        # ---------------- per query-block pipeline
        for qb in range(NQB):
            q0 = qb * P
            # P2: load + transpose query block
            qbf = sb.tile([P, C], BF16, tag="qbf")
            nc.gpsimd.dma_start(out=qbf[:], in_=query[q0:q0 + P, :])
            qT = sb.tile([P, 2, P], BF16, tag="qT")
            for half in range(2):
                tps = pp.tile([P, P], BF16, space="PSUM", tag="qtps")
                nc.tensor.transpose(
                    out=tps[:], in_=qbf[:, half * P:(half + 1) * P], identity=ident[:])
                nc.scalar.activation(out=qT[:, half, :], in_=tps[:], func=ACT_COPY)
            poff = pp.tile([P, NOFF + NATT], FP32, space="PSUM", tag="poff")
            nc.tensor.matmul(out=poff[:], lhsT=qT[:, 0, :], rhs=t_wcat[:, 0, :],
                             start=True, stop=False)
            nc.tensor.matmul(out=poff[:], lhsT=qT[:, 1, :], rhs=t_wcat[:, 1, :],
                             start=False, stop=False)
            nc.tensor.matmul(out=poff[:], lhsT=t_ones[:, :], rhs=t_bcat[:],
                             start=False, stop=True)
            # px2 = ref*dims + off(+1.5 folded in bias)   [128, 256] (h,l,p,c)
            rt = sb.tile([P, NP * 2], FP32, tag="rt")
            nc.sync.dma_start(out=rt[:], in_=refp[q0:q0 + P, :])
            refd = sb.tile([P, NOFF], FP32, tag="refd")
            r3 = rt[:].rearrange("p (q c) -> p q c", c=2).unsqueeze(1)
            nc.vector.tensor_tensor(
                out=refd[:].rearrange("p (g q c) -> p g q c", g=NH * NL, c=2),
                in0=r3.to_broadcast([P, NH * NL, NP, 2]),
                in1=t_dims[:].rearrange("p (g q c) -> p g q c", g=NH * NL, c=2),
                op=MULT)
            px2 = sb.tile([P, NOFF], FP32, tag="px2")
            nc.vector.tensor_tensor(out=px2[:], in0=refd[:], in1=poff[:, :NOFF], op=ADD)
            # softmax (no max-sub; logits are O(1))
            expl = sb.tile([P, NATT], FP32, tag="expl")
            nc.scalar.activation(out=expl[:], in_=poff[:, NOFF:], func=ACT_EXP)
            den = sb.tile([P, NH], FP32, tag="den")
            nc.vector.tensor_reduce(
                out=den[:], in_=expl[:].rearrange("p (h a) -> p h a", h=NH),
                axis=mybir.AxisListType.X, op=ADD)
            rden = sb.tile([P, NH], FP32, tag="rden")
            nc.vector.reciprocal(out=rden[:], in_=den[:])
            attw = sb.tile([P, NATT], FP32, tag="attw")
            nc.vector.tensor_tensor(
                out=attw[:].rearrange("p (h a) -> p h a", h=NH),
                in0=expl[:].rearrange("p (h a) -> p h a", h=NH),
                in1=rden[:].unsqueeze(2).to_broadcast([P, NH, NL * NP]),
                op=MULT)
            # clamp, shift, floor
            pxc = sb.tile([P, NOFF], FP32, tag="pxc")
            nc.vector.tensor_tensor(out=pxc[:], in0=px2[:], in1=t_dims2[:], op=MINO)
            pxs = sb.tile([P, NOFF], FP32, tag="pxs")
            nc.scalar.activation(out=pxs[:], in_=pxc[:], func=ACT_RELU)
            # x0s = floor(pxs) via round(pxs - 0.5) (fp32 +2^23 trick); the
            # exact-integer edge gives x0s = pxs - 1, fx = 1.0 -> same result.
            x0s = sb.tile([P, NOFF], FP32, tag="x0s")
            nc.vector.tensor_scalar(out=x0s[:], in0=pxs[:], scalar1=8388607.5,
                                    scalar2=8388608.0, op0=ADD, op1=SUB)
            fx = sb.tile([P, NOFF], FP32, tag="fx")
            nc.vector.tensor_tensor(out=fx[:], in0=pxs[:], in1=x0s[:], op=SUB)
            # corner validity
            ge0 = sb.tile([P, NOFF], FP32, tag="ge0")
            nc.vector.tensor_scalar(out=ge0[:], in0=x0s[:], scalar1=2.0, scalar2=None, op0=GE)
            v0 = sb.tile([P, NOFF], FP32, tag="v0")
            nc.vector.tensor_tensor(out=v0[:], in0=x0s[:], in1=t_dimsp1[:], op=LE)
            nc.vector.tensor_tensor(out=v0[:], in0=v0[:], in1=ge0[:], op=MULT)
            ge1 = sb.tile([P, NOFF], FP32, tag="ge1")
            nc.vector.tensor_scalar(out=ge1[:], in0=x0s[:], scalar1=1.0, scalar2=None, op0=GE)
            v1 = sb.tile([P, NOFF], FP32, tag="v1")
            nc.vector.tensor_tensor(out=v1[:], in0=x0s[:], in1=t_dims[:], op=LE)
            nc.vector.tensor_tensor(out=v1[:], in0=v1[:], in1=ge1[:], op=MULT)
            fm1 = sb.tile([P, NOFF], FP32, tag="fm1")  # (1 - fx)
            nc.vector.tensor_scalar(out=fm1[:], in0=fx[:], scalar1=1.0, scalar2=-1.0,
                                    op0=SUB, op1=MULT)
            # weight pairs: wxp [p, hlp, px], wya [p, hlp, y] (attn folded into y)
            wxp = sb.tile([P, HLP, 2], FP32, tag="wxp")
            x_of = lambda t: t[:].rearrange("p (g c) -> p g c", c=2)[:, :, 0]
            y_of = lambda t: t[:].rearrange("p (g c) -> p g c", c=2)[:, :, 1]
            nc.vector.tensor_tensor(out=wxp[:, :, 0], in0=x_of(fm1), in1=x_of(v0), op=MULT)
            nc.vector.tensor_tensor(out=wxp[:, :, 1], in0=x_of(fx), in1=x_of(v1), op=MULT)
            wya = sb.tile([P, HLP, 2], FP32, tag="wya")
            nc.vector.tensor_tensor(out=wya[:, :, 0], in0=y_of(fm1), in1=y_of(v0), op=MULT)
            nc.vector.tensor_tensor(out=wya[:, :, 1], in0=y_of(fx), in1=y_of(v1), op=MULT)
            aex = attw[:].unsqueeze(2).to_broadcast([P, HLP, 2])
            nc.vector.tensor_tensor(out=wya[:], in0=wya[:], in1=aex, op=MULT)
            # full corner weights [p, hlp, y, px]
            w4 = sb.tile([P, HLP, 2, 2], FP32, tag="w4")
            nc.vector.tensor_tensor(
                out=w4[:],
                in0=wya[:].unsqueeze(3).to_broadcast([P, HLP, 2, 2]),
                in1=wxp[:].unsqueeze(2).to_broadcast([P, HLP, 2, 2]),
                op=MULT)
            # gather row indices
            xcs = sb.tile([P, NOFF], FP32, tag="xcs")
            nc.vector.tensor_tensor(out=xcs[:], in0=x0s[:], in1=t_lowc[:], op=MAXO)
            nc.vector.tensor_tensor(out=xcs[:], in0=xcs[:], in1=t_dimsp1[:], op=MINO)
            t1 = sb.tile([P, HLP], FP32, tag="t1")
            nc.vector.tensor_tensor(out=t1[:], in0=x_of(xcs), in1=t_kk[:], op=ADD)
            ia = sb.tile([P, HLP], FP32, tag="ia")
            nc.vector.tensor_tensor(out=ia[:], in0=y_of(xcs), in1=t_wx[:], op=MULT)
            nc.vector.tensor_tensor(out=ia[:], in0=ia[:], in1=t1[:], op=ADD)
            yb = sb.tile([P, HLP], FP32, tag="yb")
            nc.vector.tensor_scalar(out=yb[:], in0=y_of(x0s), scalar1=1.0, scalar2=2.0,
                                    op0=ADD, op1=MAXO)
            nc.vector.tensor_tensor(out=yb[:], in0=yb[:], in1=y_of(t_dimsp1), op=MINO)
            ib = sb.tile([P, HLP], FP32, tag="ib")
            nc.vector.tensor_tensor(out=ib[:], in0=yb[:], in1=t_wx[:], op=MULT)
            nc.vector.tensor_tensor(out=ib[:], in0=ib[:], in1=t1[:], op=ADD)
            idxt = sb.tile([P, HLP, 2], I32, tag="idxt")
            nc.vector.tensor_copy(out=idxt[:, :, 0], in_=ia[:])
            nc.vector.tensor_copy(out=idxt[:, :, 1], in_=ib[:])

            # P4: gather + weighted reduce per head
            outq = sb.tile([P, NH, HD], FP32, tag="outq")
            for hg in range(NH // HGRP):
                gsl = pg.tile([P, HGRP * NL * NP * 2, 2 * HD], BF16, tag="gt")
                for hh in range(HGRP):
                    for lp in range(NL * NP):
                        g_idx = (hg * HGRP + hh) * NL * NP + lp
                        for y in range(2):
                            nc.gpsimd.indirect_dma_start(
                                out=gsl[:, (hh * NL * NP + lp) * 2 + y, :],
                                out_offset=None,
                                in_=vtbl_flat,
                                in_offset=bass.IndirectOffsetOnAxis(
                                    ap=idxt[:, g_idx, y:y + 1], axis=0),
                            )
                for hh in range(HGRP):
                    h = hg * HGRP + hh
                    gh = gsl[:, hh * NL * NP * 2:(hh + 1) * NL * NP * 2, :]
                    gw = sb.tile([P, NL * NP * 4, HD], FP32, tag="gw")
                    nc.vector.tensor_tensor(
                        out=gw[:],
                        in0=gh.rearrange("p s (x c) -> p (s x) c", x=2),
                        in1=w4[:, h * NL * NP:(h + 1) * NL * NP, :, :]
                            .rearrange("p a y x -> p (a y x)")
                            .unsqueeze(2).to_broadcast([P, NL * NP * 4, HD]),
                        op=MULT)
                    nc.vector.tensor_reduce(
                        out=outq[:, h, :],
                        in_=gw[:].rearrange("p s c -> p c s"),
                        axis=mybir.AxisListType.X, op=ADD)

            # P5: final projection
            oc = sb.tile([P, C], BF16, tag="oc")
            nc.vector.tensor_copy(out=oc[:], in_=outq[:].rearrange("p h c -> p (h c)"))
            oT = sb.tile([P, 2, P], BF16, tag="oT")
            for half in range(2):
                tps2 = pp.tile([P, P], BF16, space="PSUM", tag="otps")
                nc.tensor.transpose(
                    out=tps2[:], in_=oc[:, half * P:(half + 1) * P], identity=ident[:])
                nc.scalar.activation(out=oT[:, half, :], in_=tps2[:], func=ACT_COPY)
            pfin = pp.tile([P, C], FP32, space="PSUM", tag="pfin")
            nc.tensor.matmul(out=pfin[:], lhsT=oT[:, 0, :], rhs=t_wout[:, 0, :],
                             start=True, stop=False)
            nc.tensor.matmul(out=pfin[:], lhsT=oT[:, 1, :], rhs=t_wout[:, 1, :],
                             start=False, stop=False)
            nc.tensor.matmul(out=pfin[:], lhsT=t_ones[:, :], rhs=t_bout[:],
                             start=False, stop=True)
            fout = sb.tile([P, C], FP32, tag="fout")
            nc.scalar.activation(out=fout[:], in_=pfin[:], func=ACT_COPY)
            nc.sync.dma_start(out=out[q0:q0 + P, :], in_=fout[:])

        for _pool in (pp, pg, sb3, sb, cpool):
            _pool.release()

    nc.compile()
    return nc


# ================================================================ host consts
def _host_consts():
    j = np.arange(NOFF)
    li = (j // 8) % NL
    ci = j % 2
    W = np.array([w for _, w in SPATIAL_SHAPES], np.float64)
    H = np.array([h for h, _ in SPATIAL_SHAPES], np.float64)
    dim_j = np.where(ci == 0, W[li], H[li]).astype(np.float32)
    dims = np.broadcast_to(dim_j, (P, NOFF)).copy()
    dimsp1 = dims + 1.0
    dims2 = dims + 2.0
    lowc = np.broadcast_to(np.where(ci == 0, 1.0, 2.0).astype(np.float32), (P, NOFF)).copy()

    g = np.arange(HLP)
    lg = (g // 4) % NL
    hg = g // 16
    wx = W[lg].astype(np.float32)
    base = np.array(LEVEL_BASE, np.float64)
    kk = (hg * NVT + base[lg] - 2 * W[lg] - 1).astype(np.float32)
    wxc = np.broadcast_to(wx, (P, HLP)).copy()
    kkc = np.broadcast_to(kk, (P, HLP)).copy()
    onesr = np.ones((1, P), np.float32)
    return dims, dimsp1, dims2, lowc, wxc, kkc, onesr


def _prep_in_maps(inputs):
    import ml_dtypes
    bf16 = ml_dtypes.bfloat16

    value = np.asarray(inputs["value"], np.float32)
    query = np.asarray(inputs["query"], np.float32)
    refp = np.asarray(inputs["reference_points"], np.float32)
    w_off = np.asarray(inputs["w_off"], np.float32)
    b_off = np.asarray(inputs["b_off"], np.float32)
    w_attn = np.asarray(inputs["w_attn"], np.float32)
    b_attn = np.asarray(inputs["b_attn"], np.float32)
    w_val = np.asarray(inputs["w_val"], np.float32)
    b_val = np.asarray(inputs["b_val"], np.float32)
    w_out = np.asarray(inputs["w_out"], np.float32)
    b_out = np.asarray(inputs["b_out"], np.float32)

    wcat = np.concatenate([w_off, w_attn], axis=1).astype(bf16)
    bcat = np.concatenate([b_off + 1.5, b_attn])[None, :].astype(np.float32)
    wval = w_val.astype(bf16)
    bval = b_val[None, :].astype(np.float32)
    wout = w_out.astype(bf16)
    boutr = b_out[None, :].astype(np.float32)
    dims, dimsp1, dims2, lowc, wxc, kkc, onesr = _host_consts()

    qpad = np.zeros((BS, NQP, C), np.float32)
    qpad[:, :NQ] = query
    rpad = np.zeros((BS, NQP, NP * 2), np.float32)
    rpad[:, :NQ] = refp.reshape(BS, NQ, NP * 2)

    shared = dict(wcat=wcat, bcat=bcat, wval=wval, bval=bval, wout=wout,
                  bout=boutr, dims=dims, dimsp1=dimsp1, dims2=dims2,
                  lowc=lowc, wxc=wxc, kkc=kkc, onesr=onesr)
    in_maps = []
    for b in range(BS):
        m = dict(shared)
        m["value"] = np.ascontiguousarray(value[b])
        m["query"] = np.ascontiguousarray(qpad[b])
        m["refp"] = np.ascontiguousarray(rpad[b])
        in_maps.append(m)
    return in_maps


# ================================================================ entry point
def kernel(**inputs) -> np.ndarray:
    from concourse.bass_utils import run_bass_kernel_spmd

    if "nc" not in _CACHE:
        _CACHE["nc"] = _build_module()
    nc = _CACHE["nc"]

    in_maps = _prep_in_maps(inputs)
    res = run_bass_kernel_spmd(nc, in_maps, core_ids=list(range(BS)))
    out = np.stack([res.results[b]["out"][:NQ] for b in range(BS)], axis=0)
    return out.astype(np.float32)


if __name__ == "__main__":
    rng = np.random.default_rng(0)
    ins = {
        "query": rng.normal(size=(BS, NQ, C)).astype(np.float32),
        "value": rng.normal(size=(BS, NV, C)).astype(np.float32),
        "reference_points": rng.random((BS, NQ, NP, 2)).astype(np.float32),
        "w_off": (rng.normal(size=(C, NOFF)) * 0.01).astype(np.float32),
        "b_off": (rng.normal(size=(NOFF,)) * 0.5).astype(np.float32),
        "w_attn": (rng.normal(size=(C, NATT)) * C ** -0.5).astype(np.float32),
        "b_attn": np.zeros((NATT,), np.float32),
        "w_val": (rng.normal(size=(C, C)) * C ** -0.5).astype(np.float32),
        "b_val": np.zeros((C,), np.float32),
        "w_out": (rng.normal(size=(C, C)) * C ** -0.5).astype(np.float32),
        "b_out": np.zeros((C,), np.float32),
        "spatial_shapes": np.array(SPATIAL_SHAPES, np.int32),
    }
    o = kernel(**ins)
    print(o.shape, o.dtype, np.abs(o).mean())



# revision 8
# speedup vs baseline: 1.5594x; 1.5594x over previous
"""Multi-scale deformable attention on 8 Trainium2 NeuronCores.

Sharding: batch dim (8 batch elements -> 8 cores, data parallel).

Per-core pipeline:
  P1: v' = value @ w_val + b_val. Per 128-px chunk: sync-DMA load fp32,
      ACT cast -> bf16, PE transpose, PE matmul, write a 2x2-patch bf16
      table vtbl2[cell, head, 128]: cell (gy,gx) of level l holds pixels
      {gy+dy, gx+dx} (dy,dx in {0,1}) as [dy, dx, 32] = 256 B per head.
      Border cells zero-filled first (read only with weight 0).
  P2: off|attn logits = query @ [w_off|w_attn] + bias (PE, rank-1 fp32 bias).
  P3: DVE: sampling coords (shifted +2 so mod-floor is sign-safe), corner
      validity masks, bilinear*attention weights (bf16), ONE fp32 cell index
      per (q,h,l,p):  cell = y0s*(W+1) + x0s + base2 - W - 2.
  P4: fold indices into dma_gather's wrapped-int16 layout via 8 fp32 PE
      matmuls with 0/1 selection masks (partition fold q -> (q%16, q//16)),
      then 8 dma_gather calls (one per head, 2048 idxs each) fetch 256 B
      patches; DVE bf16 multiply by corner weights + strided tensor_reduce
      -> out[q, h, 32].
  P5: PE-transpose + matmul w_out + bias -> DMA out.
"""

import os
import sys

import numpy as np

if "/opt/trn_rl_repo" not in sys.path:
    sys.path.insert(0, "/opt/trn_rl_repo")
if "/opt/pypackages" not in sys.path:
    sys.path.append("/opt/pypackages")

# ---------------------------------------------------------------- problem cfg
SPATIAL_SHAPES = [(128, 128), (64, 64), (32, 32), (16, 16)]
C = 256
NH = 8
NL = 4
NP = 4
HD = C // NH          # 32
BS = 8
NQ = 2000
NQP = 2048            # query count padded to a multiple of 128
NV = sum(h * w for h, w in SPATIAL_SHAPES)   # 21760
P = 128
NQB = NQP // P        # 16 query blocks
NOFF = NH * NL * NP * 2   # 256 offset channels
NATT = NH * NL * NP       # 128 attention logits
HLP = NH * NL * NP        # 128
CELL2 = 4 * HD            # 128 elems (256 B) per (cell, head): [dy, dx, 32]
GRID = [(h + 1, w + 1) for h, w in SPATIAL_SHAPES]
BASE2 = [0]
for gh, gw in GRID[:-1]:
    BASE2.append(BASE2[-1] + gh * gw)
NCELL2 = BASE2[-1] + GRID[-1][0] * GRID[-1][1]   # 22244 (< int16 max)
NIDX = NQP // P * 0 + 2048                       # idxs per dma_gather call
QH = 8                                           # q-fold groups (128/16)

_CACHE = {}


# ================================================================ bass module
def _build_module():
    import concourse.bacc as bacc
    import concourse.bass as bass
    import concourse.mybir as mybir
    import concourse.tile as tile
    from concourse.masks import make_identity

    FP32 = mybir.dt.float32
    BF16 = mybir.dt.bfloat16
    I16 = mybir.dt.int16
    MULT = mybir.AluOpType.mult
    ADD = mybir.AluOpType.add
    SUB = mybir.AluOpType.subtract
    MAXO = mybir.AluOpType.max
    MINO = mybir.AluOpType.min
    GE = mybir.AluOpType.is_ge
    LE = mybir.AluOpType.is_le
    ACT_COPY = mybir.ActivationFunctionType.Copy
    ACT_EXP = mybir.ActivationFunctionType.Exp
    ACT_RELU = mybir.ActivationFunctionType.Relu

    nc = bacc.Bacc("TRN2", target_bir_lowering=False, debug=False)

    # ------------------------------------------------ external inputs (1 core)
    value = nc.dram_tensor("value", [NV, C], FP32, kind="ExternalInput")
    query = nc.dram_tensor("query", [NQP, C], FP32, kind="ExternalInput")
    refp = nc.dram_tensor("refp", [NQP, NP * 2], FP32, kind="ExternalInput")
    wval = nc.dram_tensor("wval", [C, C], BF16, kind="ExternalInput")
    bval = nc.dram_tensor("bval", [1, C], FP32, kind="ExternalInput")
    wcat = nc.dram_tensor("wcat", [C, NOFF + NATT], BF16, kind="ExternalInput")
    bcat = nc.dram_tensor("bcat", [1, NOFF + NATT], FP32, kind="ExternalInput")
    wout = nc.dram_tensor("wout", [C, C], BF16, kind="ExternalInput")
    bout = nc.dram_tensor("bout", [1, C], FP32, kind="ExternalInput")
    # constants, pre-replicated across partitions host-side
    dims = nc.dram_tensor("dims", [P, NOFF], FP32, kind="ExternalInput")
    dimsp1 = nc.dram_tensor("dimsp1", [P, NOFF], FP32, kind="ExternalInput")
    dims2 = nc.dram_tensor("dims2", [P, NOFF], FP32, kind="ExternalInput")
    wxc = nc.dram_tensor("wxc", [P, HLP], FP32, kind="ExternalInput")
    kkc = nc.dram_tensor("kkc", [P, HLP], FP32, kind="ExternalInput")
    efold = nc.dram_tensor("efold", [P, QH, P], FP32, kind="ExternalInput")
    onesr = nc.dram_tensor("onesr", [1, P], FP32, kind="ExternalInput")

    out = nc.dram_tensor("out", [NQP, C], FP32, kind="ExternalOutput")

    # ------------------------------------------------ internal DRAM scratch
    vtbl2 = nc.dram_tensor("vtbl2", [NCELL2, NH, CELL2], BF16)
    ROWE = NH * CELL2     # 1024 elems per cell row (all heads)

    with tile.TileContext(nc) as tc:
        cpool = tc.alloc_tile_pool(name="consts", bufs=1)
        sb = tc.alloc_tile_pool(name="sb", bufs=2)
        sb3 = tc.alloc_tile_pool(name="sb3", bufs=3)
        pg = tc.alloc_tile_pool(name="pg", bufs=2)  # gather dests
        pp1 = tc.alloc_tile_pool(name="pp1", bufs=2, space="PSUM")

        # ---------------- constants to SBUF
        t_wval = cpool.tile([P, 2, C], BF16)      # [k-half, khalf-idx, out]
        nc.sync.dma_start(out=t_wval[:], in_=wval[:].rearrange("(a k) n -> k a n", a=2))
        t_wcat = cpool.tile([P, 2, NOFF + NATT], BF16)
        nc.sync.dma_start(out=t_wcat[:], in_=wcat[:].rearrange("(a k) n -> k a n", a=2))
        t_wout = cpool.tile([P, 2, C], BF16)
        nc.sync.dma_start(out=t_wout[:], in_=wout[:].rearrange("(a k) n -> k a n", a=2))
        t_bval = cpool.tile([1, C], FP32)
        nc.sync.dma_start(out=t_bval[:], in_=bval[:])
        t_bcat = cpool.tile([1, NOFF + NATT], FP32)
        nc.sync.dma_start(out=t_bcat[:], in_=bcat[:])
        t_bout = cpool.tile([1, C], FP32)
        nc.sync.dma_start(out=t_bout[:], in_=bout[:])
        t_dims = cpool.tile([P, NOFF], FP32)
        nc.sync.dma_start(out=t_dims[:], in_=dims[:])
        t_dimsp1 = cpool.tile([P, NOFF], FP32)
        nc.sync.dma_start(out=t_dimsp1[:], in_=dimsp1[:])
        t_dims2 = cpool.tile([P, NOFF], FP32)
        nc.sync.dma_start(out=t_dims2[:], in_=dims2[:])
        t_wx = cpool.tile([P, HLP], FP32)
        nc.sync.dma_start(out=t_wx[:], in_=wxc[:])
        t_kk = cpool.tile([P, HLP], FP32)
        nc.sync.dma_start(out=t_kk[:], in_=kkc[:])
        t_ef = cpool.tile([P, QH, P], FP32)
        nc.sync.dma_start(out=t_ef[:], in_=efold[:])
        t_ones = cpool.tile([1, P], FP32)
        nc.sync.dma_start(out=t_ones[:], in_=onesr[:])
        ident = cpool.tile([P, P], BF16)
        make_identity(nc, ident[:])

        # ---------------- border cells zero-fill (read only with weight 0)
        ztile = cpool.tile([P, 1032], BF16)
        nc.vector.memset(ztile[:], 0)
        for (Hl, Wl), (GH, GW), b2 in zip(SPATIAL_SHAPES, GRID, BASE2):
            rowe = GW * ROWE // P          # elems per partition for one grid row
            for gr in (0, GH - 1):         # top/bottom grid rows, contiguous
                dst = bass.AP(vtbl2, (b2 + gr * GW) * ROWE, [[rowe, P], [1, rowe]])
                nc.sync.dma_start(out=dst, in_=ztile[:, :rowe])
            for gc in (0, GW - 1):         # left/right cols, minus corners
                dst = bass.AP(
                    vtbl2, (b2 + GW + gc) * ROWE,
                    [[GW * ROWE, GH - 2], [1, ROWE]])
                nc.sync.dma_start(out=dst, in_=ztile[:GH - 2, :ROWE])

        # ---------------- P1: load, cast, PE-transpose, project, write table
        pxbase = 0
        for (Hl, Wl), (GH, GW), b2 in zip(SPATIAL_SHAPES, GRID, BASE2):
            R = P // Wl                    # pixel rows per 128-px chunk
            for ch in range(Hl * Wl // P):
                vraw = sb3.tile([P, C], FP32, tag="vraw")
                nc.sync.dma_start(
                    out=vraw[:], in_=value[pxbase:pxbase + P, :])
                vb = sb3.tile([P, C], BF16, tag="vb")
                nc.scalar.activation(out=vb[:], in_=vraw[:], func=ACT_COPY)
                vT = sb.tile([P, 2, P], BF16, tag="vT")
                for half in range(2):
                    tps = pp1.tile([P, P], BF16, space="PSUM", tag="vtp")
                    nc.tensor.transpose(
                        out=tps[:], in_=vb[:, half * P:(half + 1) * P],
                        identity=ident[:])
                    nc.scalar.activation(out=vT[:, half, :], in_=tps[:],
                                         func=ACT_COPY)
                ps = pp1.tile([P, C], FP32, space="PSUM", tag="p1ps")
                nc.tensor.matmul(out=ps[:], lhsT=vT[:, 0, :], rhs=t_wval[:, 0, :],
                                 start=True, stop=False)
                nc.tensor.matmul(out=ps[:], lhsT=vT[:, 1, :], rhs=t_wval[:, 1, :],
                                 start=False, stop=False)
                nc.tensor.matmul(out=ps[:], lhsT=t_ones[:, :], rhs=t_bval[:],
                                 start=False, stop=True)
                ev = sb3.tile([P, C], BF16, tag="p1ev")
                nc.scalar.activation(out=ev[:], in_=ps[:], func=ACT_COPY)
                evh = ev[:].rearrange("p (h c) -> p h c", h=NH)
                y0 = ch * R
                # pixel (y,x) -> cell (y+1-dy, x+1-dx) slot (dy,dx)
                for dy in range(2):
                    for dx in range(2):
                        off0 = ((b2 + (y0 + 1 - dy) * GW + (1 - dx)) * ROWE
                                + (dy * 2 + dx) * HD)
                        dst = bass.AP(
                            vtbl2, off0,
                            [[GW * ROWE, R], [ROWE, Wl], [CELL2, NH], [1, HD]])
                        nc.sync.dma_start(out=dst, in_=evh)
                pxbase += P

        pp1.release()
        pp = tc.alloc_tile_pool(name="pp", bufs=2, space="PSUM")

        # ---------------- per query-block pipeline
        for qb in range(NQB):
            q0 = qb * P
            # P2: load + transpose query block
            qraw = sb.tile([P, C], FP32, tag="qraw")
            nc.sync.dma_start(out=qraw[:], in_=query[q0:q0 + P, :])
            qbf = sb.tile([P, C], BF16, tag="qbf")
            nc.scalar.activation(out=qbf[:], in_=qraw[:], func=ACT_COPY)
            qT = sb.tile([P, 2, P], BF16, tag="qT")
            for half in range(2):
                tps = pp.tile([P, P], BF16, space="PSUM", tag="tps")
                nc.tensor.transpose(
                    out=tps[:], in_=qbf[:, half * P:(half + 1) * P], identity=ident[:])
                nc.scalar.activation(out=qT[:, half, :], in_=tps[:], func=ACT_COPY)
            poff = pp.tile([P, NOFF + NATT], FP32, space="PSUM", tag="poff")
            nc.tensor.matmul(out=poff[:], lhsT=qT[:, 0, :], rhs=t_wcat[:, 0, :],
                             start=True, stop=False)
            nc.tensor.matmul(out=poff[:], lhsT=qT[:, 1, :], rhs=t_wcat[:, 1, :],
                             start=False, stop=False)
            nc.tensor.matmul(out=poff[:], lhsT=t_ones[:, :], rhs=t_bcat[:],
                             start=False, stop=True)
            # px2 = ref*dims + off(+1.5 folded in bias)   [128, 256] (h,l,p,c)
            rt = sb.tile([P, NP * 2], FP32, tag="rt")
            nc.sync.dma_start(out=rt[:], in_=refp[q0:q0 + P, :])
            refd = sb.tile([P, NOFF], FP32, tag="refd")
            r3 = rt[:].rearrange("p (q c) -> p q c", c=2).unsqueeze(1)
            nc.vector.tensor_tensor(
                out=refd[:].rearrange("p (g q c) -> p g q c", g=NH * NL, c=2),
                in0=r3.to_broadcast([P, NH * NL, NP, 2]),
                in1=t_dims[:].rearrange("p (g q c) -> p g q c", g=NH * NL, c=2),
                op=MULT)
            px2 = sb.tile([P, NOFF], FP32, tag="px2")
            nc.vector.tensor_tensor(out=px2[:], in0=refd[:], in1=poff[:, :NOFF], op=ADD)
            # softmax (no max-sub; logits are O(1))
            expl = sb.tile([P, NATT], FP32, tag="expl")
            nc.scalar.activation(out=expl[:], in_=poff[:, NOFF:], func=ACT_EXP)
            den = sb.tile([P, NH], FP32, tag="den")
            nc.vector.tensor_reduce(
                out=den[:], in_=expl[:].rearrange("p (h a) -> p h a", h=NH),
                axis=mybir.AxisListType.X, op=ADD)
            rden = sb.tile([P, NH], FP32, tag="rden")
            nc.vector.reciprocal(out=rden[:], in_=den[:])
            attw = sb.tile([P, NATT], FP32, tag="attw")
            nc.vector.tensor_tensor(
                out=attw[:].rearrange("p (h a) -> p h a", h=NH),
                in0=expl[:].rearrange("p (h a) -> p h a", h=NH),
                in1=rden[:].unsqueeze(2).to_broadcast([P, NH, NL * NP]),
                op=MULT)
            # clamp, shift, floor
            pxc = sb.tile([P, NOFF], FP32, tag="pxc")
            nc.vector.tensor_tensor(out=pxc[:], in0=px2[:], in1=t_dims2[:], op=MINO)
            pxs = sb.tile([P, NOFF], FP32, tag="pxs")
            nc.scalar.activation(out=pxs[:], in_=pxc[:], func=ACT_RELU)
            # x0s = floor(pxs) via round(pxs - 0.5) (fp32 +2^23 trick); the
            # exact-integer edge gives x0s = pxs - 1, fx = 1.0 -> same result.
            x0s = sb.tile([P, NOFF], FP32, tag="x0s")
            nc.vector.tensor_scalar(out=x0s[:], in0=pxs[:], scalar1=8388607.5,
                                    scalar2=8388608.0, op0=ADD, op1=SUB)
            fx = sb.tile([P, NOFF], FP32, tag="fx")
            nc.vector.tensor_tensor(out=fx[:], in0=pxs[:], in1=x0s[:], op=SUB)
            # corner validity
            ge0 = sb.tile([P, NOFF], FP32, tag="ge0")
            nc.vector.tensor_scalar(out=ge0[:], in0=x0s[:], scalar1=2.0, scalar2=None, op0=GE)
            v0 = sb.tile([P, NOFF], FP32, tag="v0")
            nc.vector.tensor_tensor(out=v0[:], in0=x0s[:], in1=t_dimsp1[:], op=LE)
            nc.vector.tensor_tensor(out=v0[:], in0=v0[:], in1=ge0[:], op=MULT)
            ge1 = sb.tile([P, NOFF], FP32, tag="ge1")
            nc.vector.tensor_scalar(out=ge1[:], in0=x0s[:], scalar1=1.0, scalar2=None, op0=GE)
            v1 = sb.tile([P, NOFF], FP32, tag="v1")
            nc.vector.tensor_tensor(out=v1[:], in0=x0s[:], in1=t_dims[:], op=LE)
            nc.vector.tensor_tensor(out=v1[:], in0=v1[:], in1=ge1[:], op=MULT)
            fm1 = sb.tile([P, NOFF], FP32, tag="fm1")  # (1 - fx)
            nc.vector.tensor_scalar(out=fm1[:], in0=fx[:], scalar1=1.0, scalar2=-1.0,
                                    op0=SUB, op1=MULT)
            # weight pairs: wxp [p, hlp, px], wya [p, hlp, y] (attn folded into y)
            wxp = sb.tile([P, HLP, 2], FP32, tag="wxp")
            x_of = lambda t: t[:].rearrange("p (g c) -> p g c", c=2)[:, :, 0]
            y_of = lambda t: t[:].rearrange("p (g c) -> p g c", c=2)[:, :, 1]
            nc.vector.tensor_tensor(out=wxp[:, :, 0], in0=x_of(fm1), in1=x_of(v0), op=MULT)
            nc.vector.tensor_tensor(out=wxp[:, :, 1], in0=x_of(fx), in1=x_of(v1), op=MULT)
            wya = sb.tile([P, HLP, 2], FP32, tag="wya")
            nc.vector.tensor_tensor(out=wya[:, :, 0], in0=y_of(fm1), in1=y_of(v0), op=MULT)
            nc.vector.tensor_tensor(out=wya[:, :, 1], in0=y_of(fx), in1=y_of(v1), op=MULT)
            aex = attw[:].unsqueeze(2).to_broadcast([P, HLP, 2])
            nc.vector.tensor_tensor(out=wya[:], in0=wya[:], in1=aex, op=MULT)
            # full corner weights [p, hlp(h,l,p), dy, dx], cast to bf16
            w4 = sb.tile([P, HLP, 2, 2], FP32, tag="w4")
            nc.vector.tensor_tensor(
                out=w4[:],
                in0=wya[:].unsqueeze(3).to_broadcast([P, HLP, 2, 2]),
                in1=wxp[:].unsqueeze(2).to_broadcast([P, HLP, 2, 2]),
                op=MULT)
            w4b = sb.tile([P, HLP, 2, 2], BF16, tag="w4b")
            nc.scalar.activation(
                out=w4b[:].rearrange("p a y x -> p (a y x)"),
                in_=w4[:].rearrange("p a y x -> p (a y x)"), func=ACT_COPY)
            # cell index: cell = y0s*(W+1) + x0s + base2 - W - 2   [P, HLP]
            xcs = sb.tile([P, NOFF], FP32, tag="xcs")
            nc.vector.tensor_scalar(out=xcs[:], in0=x0s[:], scalar1=1.0, scalar2=None,
                                    op0=MAXO)
            nc.vector.tensor_tensor(out=xcs[:], in0=xcs[:], in1=t_dimsp1[:], op=MINO)
            ia = sb.tile([P, HLP], FP32, tag="ia")
            nc.vector.tensor_tensor(out=ia[:], in0=y_of(xcs), in1=t_wx[:], op=MULT)
            nc.vector.tensor_tensor(out=ia[:], in0=ia[:], in1=x_of(xcs), op=ADD)
            nc.vector.tensor_tensor(out=ia[:], in0=ia[:], in1=t_kk[:], op=ADD)

            # fold q -> (q%16, q//16): psI[r, s] = ia[16*qh + r%16, s]
            idxall = sb.tile([P, NH, NL * NP, QH], I16, tag="idxall")
            for qh in range(QH):
                psI = pp.tile([P, HLP], FP32, space="PSUM", tag="psI")
                nc.tensor.matmul(out=psI[:], lhsT=t_ef[:, qh, :], rhs=ia[:],
                                 start=True, stop=True)
                nc.vector.tensor_copy(
                    out=idxall[:, :, :, qh],
                    in_=psI[:].rearrange("p (h a) -> p h a", h=NH))

            # P4: per-head dma_gather of 2048 256B patches
            gsl = pg.tile([P, NH, NL * NP, CELL2], BF16, tag="gt")
            for h in range(NH):
                in_ap = bass.AP(vtbl2, h * CELL2, [[ROWE, NCELL2], [1, CELL2]])
                nc.gpsimd.dma_gather(
                    out_ap=gsl[:, h, :, :],
                    in_ap=in_ap,
                    idxs_ap=idxall[:, h, :, :].rearrange("p a q -> p (a q)"),
                    num_idxs=NIDX,
                    num_idxs_reg=NIDX,
                    elem_size=CELL2,
                    elem_step=ROWE,
                    single_packet=False,
                )
            # weighted reduce per head
            outq = sb.tile([P, NH, HD], FP32, tag="outq")
            for h in range(NH):
                gw = sb.tile([P, NL * NP * 4, HD], BF16, tag="gw")
                nc.vector.tensor_tensor(
                    out=gw[:],
                    in0=gsl[:, h, :, :].rearrange("p a (s c) -> p (a s) c", s=4),
                    in1=w4b[:, h * NL * NP:(h + 1) * NL * NP, :, :]
                        .rearrange("p a y x -> p (a y x)")
                        .unsqueeze(2).to_broadcast([P, NL * NP * 4, HD]),
                    op=MULT)
                nc.vector.tensor_reduce(
                    out=outq[:, h, :],
                    in_=gw[:].rearrange("p s c -> p c s"),
                    axis=mybir.AxisListType.X, op=ADD)

            # P5: final projection
            oc = sb.tile([P, C], BF16, tag="oc")
            nc.scalar.activation(
                out=oc[:], in_=outq[:].rearrange("p h c -> p (h c)"), func=ACT_COPY)
            oT = sb.tile([P, 2, P], BF16, tag="oT")
            for half in range(2):
                tps2 = pp.tile([P, P], BF16, space="PSUM", tag="tps")
                nc.tensor.transpose(
                    out=tps2[:], in_=oc[:, half * P:(half + 1) * P], identity=ident[:])
                nc.scalar.activation(out=oT[:, half, :], in_=tps2[:], func=ACT_COPY)
            pfin = pp.tile([P, C], FP32, space="PSUM", tag="pfin")
            nc.tensor.matmul(out=pfin[:], lhsT=oT[:, 0, :], rhs=t_wout[:, 0, :],
                             start=True, stop=False)
            nc.tensor.matmul(out=pfin[:], lhsT=oT[:, 1, :], rhs=t_wout[:, 1, :],
                             start=False, stop=False)
            nc.tensor.matmul(out=pfin[:], lhsT=t_ones[:, :], rhs=t_bout[:],
                             start=False, stop=True)
            fout = sb.tile([P, C], FP32, tag="fout")
            nc.scalar.activation(out=fout[:], in_=pfin[:], func=ACT_COPY)
            nc.sync.dma_start(out=out[q0:q0 + P, :], in_=fout[:])

        for _pool in (pp, pg, sb3, sb, cpool):
            _pool.release()

    nc.compile()
    return nc


# ================================================================ host consts
def _host_consts():
    j = np.arange(NOFF)
    li = (j // 8) % NL
    ci = j % 2
    W = np.array([w for _, w in SPATIAL_SHAPES], np.float64)
    H = np.array([h for h, _ in SPATIAL_SHAPES], np.float64)
    dim_j = np.where(ci == 0, W[li], H[li]).astype(np.float32)
    dims = np.broadcast_to(dim_j, (P, NOFF)).copy()
    dimsp1 = dims + 1.0
    dims2 = dims + 2.0

    g = np.arange(HLP)
    lg = (g // 4) % NL
    wx = (W[lg] + 1.0).astype(np.float32)
    base2 = np.array(BASE2, np.float64)
    kk = (base2[lg] - W[lg] - 2.0).astype(np.float32)
    wxc = np.broadcast_to(wx, (P, HLP)).copy()
    kkc = np.broadcast_to(kk, (P, HLP)).copy()

    # fold masks: efold[q, qh, r] = 1 iff q == 16*qh + r%16
    q = np.arange(P)[:, None, None]
    qh = np.arange(QH)[None, :, None]
    r = np.arange(P)[None, None, :]
    efold = (q == 16 * qh + r % 16).astype(np.float32)

    onesr = np.ones((1, P), np.float32)
    return dims, dimsp1, dims2, wxc, kkc, efold, onesr


def _prep_in_maps(inputs):
    import ml_dtypes
    bf16 = ml_dtypes.bfloat16

    value = np.asarray(inputs["value"], np.float32)
    query = np.asarray(inputs["query"], np.float32)
    refp = np.asarray(inputs["reference_points"], np.float32)
    w_off = np.asarray(inputs["w_off"], np.float32)
    b_off = np.asarray(inputs["b_off"], np.float32)
    w_attn = np.asarray(inputs["w_attn"], np.float32)
    b_attn = np.asarray(inputs["b_attn"], np.float32)
    w_val = np.asarray(inputs["w_val"], np.float32)
    b_val = np.asarray(inputs["b_val"], np.float32)
    w_out = np.asarray(inputs["w_out"], np.float32)
    b_out = np.asarray(inputs["b_out"], np.float32)

    wcat = np.concatenate([w_off, w_attn], axis=1).astype(bf16)
    bcat = np.concatenate([b_off + 1.5, b_attn])[None, :].astype(np.float32)
    wval = w_val.astype(bf16)
    bval = b_val[None, :].astype(np.float32)
    wout = w_out.astype(bf16)
    boutr = b_out[None, :].astype(np.float32)
    dims, dimsp1, dims2, wxc, kkc, efold, onesr = _host_consts()

    qpad = np.zeros((BS, NQP, C), np.float32)
    qpad[:, :NQ] = query
    rpad = np.zeros((BS, NQP, NP * 2), np.float32)
    rpad[:, :NQ] = refp.reshape(BS, NQ, NP * 2)

    shared = dict(wcat=wcat, bcat=bcat, wval=wval, bval=bval, wout=wout,
                  bout=boutr, dims=dims, dimsp1=dimsp1, dims2=dims2,
                  wxc=wxc, kkc=kkc, efold=efold, onesr=onesr)
    in_maps = []
    for b in range(BS):
        m = dict(shared)
        m["value"] = np.ascontiguousarray(value[b])
        m["query"] = np.ascontiguousarray(qpad[b])
        m["refp"] = np.ascontiguousarray(rpad[b])
        in_maps.append(m)
    return in_maps


# ================================================================ entry point
def kernel(**inputs) -> np.ndarray:
    from concourse.bass_utils import run_bass_kernel_spmd

    if "nc" not in _CACHE:
        _CACHE["nc"] = _build_module()
    nc = _CACHE["nc"]

    in_maps = _prep_in_maps(inputs)
    res = run_bass_kernel_spmd(nc, in_maps, core_ids=list(range(BS)))
    out = np.stack([res.results[b]["out"][:NQ] for b in range(BS)], axis=0)
    return out.astype(np.float32)


if __name__ == "__main__":
    rng = np.random.default_rng(0)
    ins = {
        "query": rng.normal(size=(BS, NQ, C)).astype(np.float32),
        "value": rng.normal(size=(BS, NV, C)).astype(np.float32),
        "reference_points": rng.random((BS, NQ, NP, 2)).astype(np.float32),
        "w_off": (rng.normal(size=(C, NOFF)) * 0.01).astype(np.float32),
        "b_off": (rng.normal(size=(NOFF,)) * 0.5).astype(np.float32),
        "w_attn": (rng.normal(size=(C, NATT)) * C ** -0.5).astype(np.float32),
        "b_attn": np.zeros((NATT,), np.float32),
        "w_val": (rng.normal(size=(C, C)) * C ** -0.5).astype(np.float32),
        "b_val": np.zeros((C,), np.float32),
        "w_out": (rng.normal(size=(C, C)) * C ** -0.5).astype(np.float32),
        "b_out": np.zeros((C,), np.float32),
        "spatial_shapes": np.array(SPATIAL_SHAPES, np.int32),
    }
    o = kernel(**ins)
    print(o.shape, o.dtype, np.abs(o).mean())
